# revision 82
# baseline (speedup 1.0000x reference)
"""Self-contained Trainium2 Bass kernel for nn_MultiHeadMPSRecurrence.

Reference computation (B=2, T=4096, D_MODEL=1024, D_HIDDEN=2048, K=4):
    ih    = causal_depthwise_conv(x @ W_ih + b, conv_w, conv_b)
    gate  = sigmoid(x @ W_gate + b)
    a     = sigmoid(x @ W_decay + b)
    z     = silu(x @ W_z + b)
    alpha = (1-gate)*a ; beta = gate*ih
    h     = scan(h_t = alpha_t*h_{t-1} + beta_t)
    out   = ((z * layernorm(h)) @ W_out + b) @ out_w + b

Sharding: 8 cores = 2 batches x 4 time-chunks of 1024 steps.  Each core runs
the full pipeline on its chunk; the sequential scan is chunk-linked through
AllGathers of per-chunk (prod(alpha), last-local-state) vectors followed by an
on-device prefix combine; each tile's local scan is then patched in place with
h = h0 + cumprod(alpha) * h_in (no re-scan, no alpha/beta spill to DRAM).

Phase structure per core (channels on partitions, 16 tiles of 128ch):
  1. ih/gate/decay projections + conv + gates + local scans (tensor engine
     ~continuously busy; decay projection runs in fp8 DoubleRow at 4x).
     Boundary exchange 1 fires after 12 tiles; fixups+LN-stat matmuls for
     those tiles overlap the remaining projections.
  2. Exchange 2, then the z projection pass (silu) whose matmuls hide the
     comm latency + LN-stat finalization (PE transposes of mean/meansq rows,
     rsqrt, mean broadcast) + fixups of the last 4 tiles + the LN apply.
  3. Output matmul out = (z*hn) @ (W_out@out_w) with streamed weights,
     scaled by rstd at PSUM eviction.

Matmuls in bf16 (fp8 e4m3 DoubleRow at 4x effective rate for the gate and
decay projections, whose sigmoid outputs tolerate the quantization) with
fp32 PSUM; elementwise/scan in bf16/fp32 (scan state fp32 in-engine).
Weight DMAs are host-packed so every descriptor line is >=1KB (full DMA bus
efficiency).  Engine placement respects the real V3 ISA: Pool/GpSimd only
runs tensor_tensor + DMA + collectives; scans/stt on DVE; per-channel
scalar scaling on Act.
"""

import functools
from contextlib import ExitStack

import ml_dtypes
import numpy as np

import concourse.tile as tile
from concourse import bacc, mybir
from concourse.bass_utils import run_bass_kernel_spmd

BF16 = mybir.dt.bfloat16
F32 = mybir.dt.float32
FP8 = mybir.dt.float8e4
AF = mybir.ActivationFunctionType
OP = mybir.AluOpType
AX = mybir.AxisListType
PM = mybir.MatmulPerfMode

B, T, D, H = 2, 4096, 1024, 2048
NCORES = 8
KCHUNKS = 4          # time chunks per batch
TC = T // KCHUNKS    # 1024 timesteps per core
HALO = 3             # conv taps reaching back in time (K-1)
TCX = TC + HALO      # 1027
NCT = H // 128       # 16 channel tiles
NDT = D // 128       # 8 d_model tiles
LN_EPS = 1e-5
SPLIT = 12           # channel tiles covered by the first boundary exchange

# Which of the phase-1 projections (0=ih, 1=gate, 2=decay) run in fp8
# DoubleRow mode.  Gate/decay feed sigmoids (error-tolerant: measured
# rel_err 0.0153 vs the 2e-2 budget); ih feeds the state directly and the
# z/output paths scale the output directly, so those stay bf16.
FP8_GATE = True
FP8_DECAY = True
S_GATE = 64.0        # fp8 weight pre-scale (keeps w*S in e4m3 normal range)
S_DEC = 128.0


def _ms_groups(fp8_gate, fp8_decay):
    ms_bf = [0] + ([] if fp8_gate else [1])
    ms_f8 = ([1] if fp8_gate else []) + ([2] if fp8_decay else [])
    if not fp8_decay:
        ms_bf.append(2)
    return ms_bf, ms_f8


def _build_program(has_out_bias: bool, has_ln_b: bool, sim_no_cc: bool = False,
                   repeat: int = 1, fp8_gate: bool = FP8_GATE,
                   fp8_decay: bool = FP8_DECAY):
    ms_bf, ms_f8 = _ms_groups(fp8_gate, fp8_decay)
    nbf, nf8 = len(ms_bf), len(ms_f8)

    nc = bacc.Bacc("TRN2", target_bir_lowering=False, debug=False,
                   enable_asserts=True, num_devices=NCORES)

    # ---------------- I/O ----------------
    xT = nc.dram_tensor("xT", [D, TCX], BF16, kind="ExternalInput").ap()
    if nf8:
        x8T = nc.dram_tensor("x8T", [D, TC], FP8, kind="ExternalInput").ap()
    # weights packed so each partition's line is contiguous in DRAM:
    # wb[ct*128+p, (mi dt c)], w8[ct*128+p, (mi dtp i c)], wz[ct*128+p, (dt c)]
    wb = nc.dram_tensor("wb", [NCT * 128, nbf * NDT * 128], BF16,
                        kind="ExternalInput").ap()
    if nf8:
        w8 = nc.dram_tensor("w8", [NCT * 128, nf8 * NDT * 128], FP8,
                            kind="ExternalInput").ap()
    wz = nc.dram_tensor("wz", [NCT * 128, NDT * 128], BF16,
                        kind="ExternalInput").ap()
    wcomb = nc.dram_tensor("wcomb", [H, D], BF16, kind="ExternalInput").ap()
    # packed per-channel vectors: [128, *] f32
    # cols: bias_ih(16) bias_halo(16) bias_gneg(16) bias_dec(16) bias_z(16)
    #       conv_b(16) ln_g(16) convw(64) onehot(8) [ln_b(16)]
    NCONST = 7 * NCT + 4 * NCT + NCORES + (NCT if has_ln_b else 0)
    cvec = nc.dram_tensor("cvec", [128, NCONST], F32, kind="ExternalInput").ap()
    if has_out_bias:
        out_bias_d = nc.dram_tensor("out_bias", [128, D], BF16,
                                    kind="ExternalInput").ap()
    out = nc.dram_tensor("out", [TC, D], F32, kind="ExternalOutput").ap()

    with tile.TileContext(nc) as tc, ExitStack() as ctx:
        dram = ctx.enter_context(tc.tile_pool(name="dram", bufs=1, space="DRAM"))

        # ---------------- constants / small resident ----------------
        consts = ctx.enter_context(tc.tile_pool(name="consts", bufs=1))
        call = consts.tile([128, NCONST], F32, tag="call")
        nc.gpsimd.dma_start(call[:], cvec[:, :])
        off = 0
        def cslice(n):
            nonlocal off
            s = call[:, off:off + n]
            off += n
            return s
        bih_s = cslice(NCT)
        bhalo_s = cslice(NCT)
        bgn_s = cslice(NCT)
        bdec_s = cslice(NCT)
        bz_s = cslice(NCT)
        cb_s = cslice(NCT)
        lng_s = cslice(NCT)
        cw_s = cslice(4 * NCT)
        oh_s = cslice(NCORES)
        if has_ln_b:
            lnb_s = cslice(NCT)
        if has_out_bias:
            obias_s = consts.tile([128, D], BF16, tag="obias")
            nc.sync.dma_start(obias_s[:], out_bias_d[:, :])
        ones_stat = consts.tile([128, 1], BF16)
        nc.vector.memset(ones_stat[:], 1.0)
        ones_bc = consts.tile([1, 128], F32)
        nc.vector.memset(ones_bc[:], 1.0)
        eps_t = consts.tile([128, 1], F32)
        nc.vector.memset(eps_t[:], LN_EPS)
        ident1 = consts.tile([1, 1], F32)
        nc.vector.memset(ident1[:], 1.0)

        # ---------------- big resident tiles ----------------
        res = ctx.enter_context(tc.tile_pool(name="res", bufs=1))
        xT_s = res.tile([128, NDT * TCX], BF16)
        if nf8:
            x8_s = res.tile([128, NDT * TC], FP8)
            x8v = x8_s[:].rearrange("p (dt t) -> p dt t", t=TC)
        z_s = res.tile([128, NCT * TC], BF16)
        h_s = res.tile([128, NCT * TC], BF16)
        cumA_s = res.tile([128, NCT * TC], BF16)
        Pbuf = res.tile([128, NCT], F32)
        Lbuf = res.tile([128, NCT], F32)
        h_in = res.tile([128, NCT], F32)
        Pall = res.tile([128, NCORES * SPLIT], F32)
        Lall = res.tile([128, NCORES * SPLIT], F32)
        Hp = res.tile([128, NCORES * SPLIT], F32)
        mu_sb = res.tile([128, TC], BF16)
        mua_sb = res.tile([1, TC], F32)       # mean row
        mub_sb = res.tile([1, TC], F32)       # meansq row
        rstd_col = res.tile([128, NDT], F32)
        var_col = res.tile([128, NDT], F32)
        tp_sb = res.tile([128, 2 * NDT], F32)
        # resident second-half output weight (cg=1), loaded during the z pass
        wres1 = res.tile([128, NCT * 512], BF16)
        if has_ln_b:
            sd_sb = res.tile([128, TC], BF16)
            sd_col = res.tile([128, NDT], F32)

        # (x loads are emitted inside phase 1, interleaved with the first
        # weight chunks so the DMA-device FIFO order matches consumption)

        def emit_body():
            n1, n2 = SPLIT, NCT - SPLIT
            cc1_in = dram.tile([1, 2 * n1 * 128], F32, tag="cc1i")
            cc1_out = dram.tile([NCORES, 2 * n1 * 128], F32,
                                addr_space="Shared", tag="cc1o")
            cc2_in = dram.tile([1, 2 * n2 * 128], F32, tag="cc2i")
            cc2_out = dram.tile([NCORES, 2 * n2 * 128], F32,
                                addr_space="Shared", tag="cc2o")

            def comm(lo, hi, cci, cco, veng):
                """AllGather (P,L) for channel tiles [lo,hi); prefix-combine
                and select this core's incoming state into h_in[:, lo:hi].

                The pack/unpack DMAs are deliberately kept OFF the SP queue:
                SP serves the weight streams and an in-order SP queue would
                head-of-line block them behind the comm's data dependency."""
                n = hi - lo
                nc.gpsimd.dma_start(
                    cci[0:1, 0:n * 128].rearrange("o (p c) -> p (o c)", p=128),
                    Pbuf[:, lo:hi])
                nc.gpsimd.dma_start(
                    cci[0:1, n * 128:2 * n * 128]
                    .rearrange("o (p c) -> p (o c)", p=128),
                    Lbuf[:, lo:hi])
                if sim_no_cc:
                    # TimelineSim can't model collectives; local DMA stand-in.
                    nc.gpsimd.dma_start(cco[0:1, :], cci[:, :])
                else:
                    nc.gpsimd.collective_compute(
                        "AllGather", OP.bypass,
                        replica_groups=[list(range(NCORES))],
                        ins=[cci.opt()], outs=[cco.opt()])
                nc.gpsimd.dma_start(
                    Pall[:, 0:NCORES * n].rearrange("p (j c) -> p j c",
                                                    j=NCORES),
                    cco[:, 0:n * 128].rearrange("j (p c) -> p j c", p=128))
                nc.gpsimd.dma_start(
                    Lall[:, 0:NCORES * n].rearrange("p (j c) -> p j c",
                                                    j=NCORES),
                    cco[:, n * 128:2 * n * 128]
                    .rearrange("j (p c) -> p j c", p=128))
                for j in range(NCORES):
                    sj = slice(j * n, (j + 1) * n)
                    sjm = slice((j - 1) * n, j * n)
                    if j % KCHUNKS == 0:
                        veng.tensor_copy(Hp[:, sj], Lall[:, sj])
                    else:
                        veng.tensor_tensor(Hp[:, sj], Pall[:, sj],
                                           Hp[:, sjm], OP.mult)
                        veng.tensor_tensor(Hp[:, sj], Hp[:, sj],
                                           Lall[:, sj], OP.add)
                veng.memset(h_in[:, lo:hi], 0.0)
                for j in range(NCORES):
                    veng.scalar_tensor_tensor(
                        h_in[:, lo:hi], Hp[:, j * n:(j + 1) * n],
                        oh_s[:, j:j + 1], h_in[:, lo:hi], OP.mult, OP.add)

            # fixup tiles 0..SPLIT-1 distributed over phase-1 iterations
            # 13..15 (cc1 lands during iteration 12; prep at iteration START
            # so the elementwise queues never stall PE's stat matmuls)
            p3_sched = {13: [0, 1, 2, 3], 14: [4, 5, 6, 7], 15: [8, 9, 10, 11]}

            # The z-pass weight pool, the matmul PSUM pool, and the apply
            # temp pool are hoisted BEFORE the phase-1 pools: their ranges
            # stay disjoint from (or shared-by-ring with) phase-1 buffers,
            # so the z pass starts without waiting for phase 1 to drain.
            with tc.tile_pool(name="pzw", bufs=3) as zwpool, \
                 tc.tile_pool(name="mmps", bufs=4, space="PSUM") as ppool, \
                 tc.tile_pool(name="pzt", bufs=2) as ztpool, \
                 tc.tile_pool(name="stps", bufs=1, space="PSUM") as spool, \
                 tc.tile_pool(name="p3b", bufs=4) as p3pool:
                psum_sh = spool.tile([1, TC], F32)
                psum_sq = spool.tile([1, TC], F32)

                h2_tiles = {}

                def fix_prep(ct, eng):
                    """h += cumA*h_in for this tile (no PE work)."""
                    hsl = h_s[:, ct * TC:(ct + 1) * TC]
                    casl = cumA_s[:, ct * TC:(ct + 1) * TC]
                    eng.scalar_tensor_tensor(
                        hsl, casl, h_in[:, ct:ct + 1], hsl, OP.mult, OP.add)

                def square_tile(ct, on_act):
                    hsl = h_s[:, ct * TC:(ct + 1) * TC]
                    h2 = p3pool.tile([128, TC], BF16, tag="h2")
                    if on_act:
                        nc.scalar.square(h2[:], hsl)
                    else:
                        nc.vector.tensor_tensor(h2[:], hsl, hsl, OP.mult)
                    h2_tiles[ct] = h2

                def fix_stats(ct):
                    """LN-stat matmuls for a prepped tile."""
                    h2 = h2_tiles.pop(ct)
                    for half in range(2):
                        nc.tensor.matmul(
                            psum_sh[0:1, half * 512:(half + 1) * 512],
                            ones_stat[:, 0:1],
                            h_s[:, ct * TC + half * 512:
                                ct * TC + (half + 1) * 512],
                            start=(ct == 0), stop=(ct == NCT - 1))
                        nc.tensor.matmul(
                            psum_sq[0:1, half * 512:(half + 1) * 512],
                            ones_stat[:, 0:1],
                            h2[:, half * 512:(half + 1) * 512],
                            start=(ct == 0), stop=(ct == NCT - 1))

                # ===== phase 1: ih/gate/decay matmuls + gates + local scans ===
                with tc.tile_pool(name="p1w", bufs=2) as wpool, \
                     tc.tile_pool(name="p1f", bufs=2) as fpool, \
                     tc.tile_pool(name="p1b", bufs=2) as bpool:

                    def load_w(ct):
                        wt = wpool.tile([128, nbf * NDT * 128], BF16, tag="wt")
                        nc.sync.dma_start(wt[:], wb[ct * 128:(ct + 1) * 128, :])
                        w8t = None
                        if nf8:
                            w8t = wpool.tile([128, nf8 * NDT * 128], FP8,
                                             tag="w8t")
                            nc.sync.dma_start(w8t[:],
                                              w8[ct * 128:(ct + 1) * 128, :])
                        return wt, w8t

                    # startup loads, interleaved in consumption order: the
                    # DMA device drains in trigger order, so each dependency
                    # lands just before its matmuls need it.
                    def xT_load(dt):
                        nc.sync.dma_start(
                            xT_s[:, dt * TCX:(dt + 1) * TCX],
                            xT[dt * 128:(dt + 1) * 128, :].rearrange(
                                "(o p) t -> p (o t)", p=128))

                    wt0 = wpool.tile([128, nbf * NDT * 128], BF16, tag="wt")
                    xT_load(0)
                    nc.sync.dma_start(wt0[:, 0:NDT * 128],
                                      wb[0:128, 0:NDT * 128])
                    for dt in range(1, NDT):
                        xT_load(dt)
                    if nbf > 1:
                        nc.sync.dma_start(wt0[:, NDT * 128:],
                                          wb[0:128, NDT * 128:])
                    w80 = None
                    if nf8:
                        for dtp in range(4):
                            nc.sync.dma_start(
                                x8v[:, 2 * dtp:2 * dtp + 2, :],
                                x8T[2 * dtp * 128:(2 * dtp + 2) * 128, :]
                                .rearrange("(dt p) t -> p dt t", p=128))
                        w80 = wpool.tile([128, nf8 * NDT * 128], FP8,
                                         tag="w8t")
                        nc.sync.dma_start(w80[:], w8[0:128, :])
                    wz_next = zwpool.tile([128, NDT * 128], BF16, tag="wzt")
                    nc.sync.dma_start(wz_next[:], wz[0:128, :])

                    w_next = (wt0, w80)
                    for ct in range(NCT):
                        wt, w8t = w_next
                        if ct + 1 < NCT:
                            w_next = load_w(ct + 1)
                        # prep the scheduled cc1 tiles first: their inputs
                        # have been ready since cc1, and prepping here keeps
                        # their stat matmuls (end of iteration) off the
                        # critical path of the engine queues
                        for c3 in p3_sched.get(ct, []):
                            fix_prep(c3, nc.vector)

                        def mm_bf(mi, lo, n):
                            ps = ppool.tile([128, 512], F32, tag="ps")
                            for dt in range(NDT):
                                nc.tensor.matmul(
                                    ps[:, 0:n],
                                    wt[:, (mi * NDT + dt) * 128:
                                       (mi * NDT + dt + 1) * 128],
                                    xT_s[:, dt * TCX + lo: dt * TCX + lo + n],
                                    start=(dt == 0), stop=(dt == NDT - 1),
                                )
                            return ps

                        def mm_f8(mi, lo):
                            # [128,512] psum accumulated as 2 x 256-col
                            # DoubleRow chains over 4 k-pair tiles (x8 has no
                            # halo; lo is relative to chunk start)
                            ps = ppool.tile([128, 512], F32, tag="ps")
                            for cchunk in range(2):
                                t0 = lo + cchunk * 256
                                for dtp in range(4):
                                    base = (mi * 4 + dtp) * 256
                                    nc.tensor.matmul(
                                        ps[:, cchunk * 256:(cchunk + 1) * 256],
                                        w8t[:, base:base + 256].rearrange(
                                            "p (i c) -> p i c", i=2),
                                        x8v[:, 2 * dtp:2 * dtp + 2, t0:t0 + 256],
                                        start=(dtp == 0), stop=(dtp == 3),
                                        perf_mode=PM.DoubleRow,
                                    )
                            return ps

                        # --- m0: ih over all 1027 halo columns (bf16) ---
                        ih_pre = fpool.tile([128, TCX], BF16, tag="ihpre")
                        for (lo, n, bias) in ((0, HALO, bhalo_s),
                                              (HALO, 512, bih_s),
                                              (HALO + 512, 512, bih_s)):
                            ps = mm_bf(0, lo, n)
                            nc.scalar.activation(ih_pre[:, lo:lo + n],
                                                 ps[:, 0:n], AF.Identity,
                                                 bias=bias[:, ct:ct + 1])
                        # --- m1: gp = sigmoid(-(x@Wg + bg)) = 1-gate ---
                        gp = fpool.tile([128, TC], BF16, tag="gp")
                        for half in range(2):
                            if fp8_gate:
                                ps = mm_f8(0, half * 512)
                                sc = -1.0 / S_GATE
                            else:
                                ps = mm_bf(1, HALO + half * 512, 512)
                                sc = -1.0
                            nc.scalar.activation(
                                gp[:, half * 512:(half + 1) * 512], ps[:, :],
                                AF.Sigmoid, bias=bgn_s[:, ct:ct + 1], scale=sc)
                        # --- m2: a = sigmoid(x@Wd + bd) ---
                        a_t = fpool.tile([128, TC], BF16, tag="a")
                        for half in range(2):
                            if fp8_decay:
                                ps = mm_f8(nf8 - 1, half * 512)
                                sc = 1.0 / S_DEC
                            else:
                                ps = mm_bf(nbf - 1, HALO + half * 512, 512)
                                sc = 1.0
                            nc.scalar.activation(
                                a_t[:, half * 512:(half + 1) * 512], ps[:, :],
                                AF.Sigmoid, bias=bdec_s[:, ct:ct + 1], scale=sc)

                        # --- causal depthwise conv (4 taps, halo in ih_pre) ---
                        # conv: init via DVE tensor_scalar, taps 2/1 via
                        # Act per-channel scale + Pool add (Pool's real ISA
                        # only has tensor_tensor), tap 0 as DVE stt
                        ihc = fpool.tile([128, TC], BF16, tag="ihc")
                        nc.vector.tensor_scalar(
                            ihc[:], ih_pre[:, 3:3 + TC],
                            cw_s[:, ct * 4 + 3:ct * 4 + 4],
                            cb_s[:, ct:ct + 1], OP.mult, OP.add)
                        for j in (2, 1):
                            tsc = fpool.tile([128, TC], BF16, tag="tsc")
                            nc.scalar.activation(
                                tsc[:], ih_pre[:, j:j + TC], AF.Identity,
                                bias=0.0,
                                scale=cw_s[:, ct * 4 + j:ct * 4 + j + 1])
                            nc.gpsimd.tensor_tensor(ihc[:], tsc[:], ihc[:],
                                                    OP.add)
                        nc.vector.scalar_tensor_tensor(
                            ihc[:], ih_pre[:, 0:TC],
                            cw_s[:, ct * 4:ct * 4 + 1],
                            ihc[:], OP.mult, OP.add)

                        # --- alpha = gp*a ; beta = (1-gp)*ihc = ihc - gp*ihc
                        # (all bf16 tensor_tensor: DVE runs them at 2x) ---
                        alpha_t = bpool.tile([128, TC], BF16, tag="alpha")
                        nc.vector.tensor_tensor(alpha_t[:], gp[:], a_t[:],
                                                OP.mult)
                        gb_t = fpool.tile([128, TC], BF16, tag="tsc")
                        nc.vector.tensor_tensor(gb_t[:], gp[:], ihc[:],
                                                OP.mult)
                        beta_t = bpool.tile([128, TC], BF16, tag="beta")
                        nc.vector.tensor_tensor(beta_t[:], ihc[:], gb_t[:],
                                                OP.subtract)

                        # --- local scans: h0 and cumprod(alpha) ---
                        hsl = h_s[:, ct * TC:(ct + 1) * TC]
                        casl = cumA_s[:, ct * TC:(ct + 1) * TC]
                        nc.vector.tensor_tensor_scan(hsl, alpha_t[:], beta_t[:],
                                                     0.0, OP.mult, OP.add)
                        nc.vector.tensor_tensor_scan(casl, alpha_t[:],
                                                     alpha_t[:], 1.0,
                                                     OP.mult, OP.bypass)
                        nc.vector.tensor_copy(Lbuf[:, ct:ct + 1],
                                              hsl[:, TC - 1:TC])
                        nc.vector.tensor_copy(Pbuf[:, ct:ct + 1],
                                              casl[:, TC - 1:TC])

                        if ct == SPLIT - 1:
                            comm(0, SPLIT, cc1_in, cc1_out, nc.vector)

                # ===== exchange 2 + z pass (hides comm + LN finalize) =========
                comm(SPLIT, NCT, cc2_in, cc2_out, nc.vector)

                if True:

                    def zproj(ct, wzt):
                        for half in range(2):
                            ps = ppool.tile([128, 512], F32, tag="ps")
                            for dt in range(NDT):
                                nc.tensor.matmul(
                                    ps[:],
                                    wzt[:, dt * 128:(dt + 1) * 128],
                                    xT_s[:, dt * TCX + HALO + half * 512:
                                         dt * TCX + HALO + (half + 1) * 512],
                                    start=(dt == 0), stop=(dt == NDT - 1))
                            nc.scalar.activation(
                                z_s[:, ct * TC + half * 512:
                                    ct * TC + (half + 1) * 512],
                                ps[:], AF.Silu, bias=bz_s[:, ct:ct + 1])

                    def stats_finalize():
                        # mean/meansq rows, PE transpose to column layout,
                        # rstd, and the mean broadcast for the LN apply.
                        nc.scalar.activation(mua_sb[0:1, :], psum_sh[:],
                                             AF.Copy, scale=1.0 / H)
                        nc.scalar.activation(mub_sb[0:1, :], psum_sq[:],
                                             AF.Copy, scale=1.0 / H)
                        tp = ppool.tile([128, 512], F32, tag="ps")
                        for j in range(NDT):
                            nc.tensor.transpose(
                                tp[:, j:j + 1],
                                mua_sb[0:1, j * 128:(j + 1) * 128],
                                ident1[:, :])
                            nc.tensor.transpose(
                                tp[:, NDT + j:NDT + j + 1],
                                mub_sb[0:1, j * 128:(j + 1) * 128],
                                ident1[:, :])
                        nc.vector.tensor_copy(tp_sb[:], tp[:, 0:2 * NDT])
                        nc.vector.tensor_tensor(var_col[:], tp_sb[:, 0:NDT],
                                                tp_sb[:, 0:NDT], OP.mult)
                        nc.vector.tensor_tensor(var_col[:],
                                                tp_sb[:, NDT:2 * NDT],
                                                var_col[:], OP.subtract)
                        nc.scalar.activation(var_col[:], var_col[:], AF.Sqrt,
                                             bias=eps_t[:, 0:1])
                        nc.vector.reciprocal(rstd_col[:], var_col[:])
                        for half in range(2):
                            mu_ps = ppool.tile([128, 512], F32, tag="ps")
                            nc.tensor.matmul(
                                mu_ps[:], ones_bc[0:1, :],
                                mua_sb[0:1, half * 512:(half + 1) * 512])
                            nc.scalar.activation(
                                mu_sb[:, half * 512:(half + 1) * 512],
                                mu_ps[:], AF.Copy)
                        if has_ln_b:
                            # sd broadcast for the ln_b term: z*(...+ln_b*sd),
                            # so the rstd at evict cancels to ln_b exactly.
                            sd_row = p3pool.tile([1, TC], F32, tag="sdrow")
                            nc.vector.tensor_tensor(
                                sd_row[:], mua_sb[0:1, :], mua_sb[0:1, :],
                                OP.mult)
                            nc.vector.tensor_tensor(
                                sd_row[:], mub_sb[0:1, :], sd_row[:],
                                OP.subtract)
                            nc.scalar.activation(sd_row[:], sd_row[:],
                                                 AF.Sqrt, bias=LN_EPS)
                            sd_ps0 = ppool.tile([128, 512], F32, tag="ps")
                            sd_ps1 = ppool.tile([128, 512], F32, tag="ps")
                            for half, sdp in enumerate((sd_ps0, sd_ps1)):
                                nc.tensor.matmul(
                                    sdp[:], ones_bc[0:1, :],
                                    sd_row[0:1, half * 512:(half + 1) * 512])
                                nc.scalar.activation(
                                    sd_sb[:, half * 512:(half + 1) * 512],
                                    sdp[:], AF.Copy)

                    def apply_ct(ct):
                        sl = slice(ct * TC, (ct + 1) * TC)
                        t1 = ztpool.tile([128, TC], BF16, tag="t1")
                        nc.vector.tensor_tensor(t1[:], h_s[:, sl], mu_sb[:],
                                                OP.subtract)
                        if has_ln_b:
                            t2 = ztpool.tile([128, TC], BF16, tag="t2")
                            nc.vector.scalar_tensor_tensor(
                                t2[:], sd_sb[:], lnb_s[:, ct:ct + 1],
                                t1[:], OP.mult, OP.bypass)
                            nc.vector.scalar_tensor_tensor(
                                t1[:], t1[:], lng_s[:, ct:ct + 1], t2[:],
                                OP.mult, OP.add)
                            nc.vector.tensor_tensor(z_s[:, sl], t1[:],
                                                    z_s[:, sl], OP.mult)
                        else:
                            nc.vector.scalar_tensor_tensor(
                                z_s[:, sl], t1[:], lng_s[:, ct:ct + 1],
                                z_s[:, sl], OP.mult, OP.mult)

                    # PE is in-order, so each iteration leads with its
                    # independent zproj matmuls; the exchange-dependent
                    # squares/stat-matmuls slot in behind enough zproj cover
                    # that their (slower) producers never stall PE.  Squares
                    # are emitted BEFORE that iteration's silu evicts so the
                    # Act queue serves them while silu still waits on PE.
                    napply = 0
                    for ct in range(NCT):
                        wzt = wz_next
                        if ct + 1 < NCT:
                            wz_next = zwpool.tile([128, NDT * 128], BF16,
                                                  tag="wzt")
                            nc.sync.dma_start(
                                wz_next[:],
                                wz[(ct + 1) * 128:(ct + 2) * 128, :])
                        if ct == 1:
                            for k in range(0, 4):
                                square_tile(k, on_act=(k % 2 == 0))
                        elif ct == 2:
                            for k in range(NCT - SPLIT):
                                fix_prep(SPLIT + k, nc.vector)
                        zproj(ct, wzt)
                        if 2 <= ct <= 4:
                            for k in range(4 * (ct - 2), 4 * (ct - 1)):
                                fix_stats(k)
                            for k in range(4 * (ct - 1), 4 * ct):
                                square_tile(k, on_act=(k % 2 == 0))
                        elif ct == 5:
                            for k in range(SPLIT, NCT):
                                fix_stats(k)
                        elif ct == 6:
                            stats_finalize()
                        elif ct >= 7:
                            apply_ct(napply)
                            napply += 1
                        if ct == 8:
                            # cg=1 output weights (resident): transfer lands
                            # on the DMA device amid the wz stream's slack
                            nc.sync.dma_start(
                                wres1[:].rearrange("p (ct c) -> p ct c",
                                                   c=512),
                                wcomb[:, 512:1024]
                                .rearrange("(ct p) c -> p ct c", p=128))
                    while napply < NCT:
                        apply_ct(napply)
                        napply += 1

            # ===== output matmul: out = (z*hn) @ wcomb, rstd at evict ========
            with tc.tile_pool(name="yw", bufs=3) as ywpool, \
                 tc.tile_pool(name="yps", bufs=8, space="PSUM") as ypool, \
                 tc.tile_pool(name="yo", bufs=4) as opool:

                def evict_y(tt, cg, y_ps):
                    out_sb = opool.tile([128, 512], F32, tag="osb")
                    nc.scalar.activation(out_sb[:], y_ps[:], AF.Copy,
                                         scale=rstd_col[:, tt:tt + 1])
                    if has_out_bias:
                        nc.vector.tensor_tensor(
                            out_sb[:], out_sb[:],
                            obias_s[:, cg * 512:(cg + 1) * 512], OP.add)
                    nc.sync.dma_start(
                        out[tt * 128:(tt + 1) * 128,
                            cg * 512:(cg + 1) * 512], out_sb[:])

                # cg=0: ct-major over 8 accumulation banks, weights streamed
                # per-ct (on the gpsimd queue: the SP queue is still draining
                # wz-ring waits and would delay the stream).
                ytiles = []
                for _tt in range(NDT):
                    ybank = ypool.tile([128, 512], F32, tag="y")
                    ytiles.append(ybank)
                for ct in range(NCT):
                    wct = ywpool.tile([128, 512], BF16, tag="wct")
                    nc.gpsimd.dma_start(
                        wct[:], wcomb[ct * 128:(ct + 1) * 128, 0:512])
                    for tt in range(NDT):
                        nc.tensor.matmul(
                            ytiles[tt][:],
                            z_s[:, ct * TC + tt * 128: ct * TC + (tt + 1) * 128],
                            wct[:],
                            start=(ct == 0), stop=(ct == NCT - 1))
                for tt in range(NDT):
                    evict_y(tt, 0, ytiles[tt])

                # cg=1: tt-major against the resident weight half, one bank
                # at a time; eviction of bank k overlaps chain k+1, and the
                # last chain's bank drains right at the program end.
                for tt in range(NDT):
                    y_ps = ypool.tile([128, 512], F32, tag="y")
                    for ct in range(NCT):
                        nc.tensor.matmul(
                            y_ps[:],
                            z_s[:, ct * TC + tt * 128: ct * TC + (tt + 1) * 128],
                            wres1[:, ct * 512:(ct + 1) * 512],
                            start=(ct == 0), stop=(ct == NCT - 1))
                    evict_y(tt, 1, y_ps)

        for _rep in range(repeat):
            emit_body()

    nc.compile()
    return nc


@functools.lru_cache(maxsize=4)
def _program(flags):
    return _build_program(*flags)


def _v2m(v):
    return np.asarray(v, np.float32).reshape(NCT, 128).T


def kernel(x, W_ih_w, W_ih_b, W_gate_w, W_gate_b, W_decay_w, W_decay_b,
           W_z_w, W_z_b, conv_w, conv_b, ln_g, ln_b, W_out_w, W_out_b,
           out_w, out_b):
    f32 = np.float32
    bf = ml_dtypes.bfloat16
    f8 = ml_dtypes.float8_e4m3
    x = np.asarray(x, f32)

    out_bias_eff = (np.asarray(W_out_b, f32) @ np.asarray(out_w, f32)
                    + np.asarray(out_b, f32))
    has_ob = bool(np.any(out_bias_eff != 0.0))
    has_lb = bool(np.any(np.asarray(ln_b) != 0.0))
    nc = _program((has_ob, has_lb))
    ms_bf, ms_f8 = _ms_groups(FP8_GATE, FP8_DECAY)

    Wm = [np.asarray(W_ih_w, f32), np.asarray(W_gate_w, f32),
          np.asarray(W_decay_w, f32)]
    scales = {1: S_GATE, 2: S_DEC}

    # wb[ct*128+p, (mi dt c)] = Wm[dt*128+p, ct*128+c]
    def pack_bf(ms):
        stack = np.stack([Wm[m] for m in ms], 0)  # [nm, D, H]
        r = stack.reshape(len(ms), NDT, 128, NCT, 128)
        return np.ascontiguousarray(
            r.transpose(3, 2, 0, 1, 4).reshape(NCT * 128, len(ms) * NDT * 128)
        ).astype(bf)

    def pack_f8(ms):
        stack = np.stack([Wm[m] * scales[m] for m in ms], 0)
        r = stack.reshape(len(ms), 4, 2, 128, NCT, 128)  # [nm, dtp, i, p, ct, c]
        return np.ascontiguousarray(
            r.transpose(4, 3, 0, 1, 2, 5).reshape(NCT * 128, len(ms) * NDT * 128)
        ).astype(f8)

    wb_m = pack_bf(ms_bf)
    wz_m = np.ascontiguousarray(
        np.asarray(W_z_w, f32).reshape(NDT, 128, NCT, 128)
        .transpose(2, 1, 0, 3).reshape(NCT * 128, NDT * 128)).astype(bf)
    wcomb_m = (np.asarray(W_out_w, f32) @ np.asarray(out_w, f32)).astype(bf)

    convw_m = (np.asarray(conv_w, f32).reshape(NCT, 128, 4)
               .transpose(1, 0, 2).reshape(128, NCT * 4))
    bias_ih_m = _v2m(W_ih_b)
    zero_halo = np.zeros((128, NCT), f32)

    shared = dict(wb=wb_m, wz=wz_m, wcomb=wcomb_m)
    if ms_f8:
        shared["w8"] = pack_f8(ms_f8)
    if has_ob:
        shared["out_bias"] = np.ascontiguousarray(
            np.tile(out_bias_eff[None, :], (128, 1)).astype(bf))

    in_maps = []
    for c in range(NCORES):
        b, k = divmod(c, KCHUNKS)
        t0 = k * TC
        if k == 0:
            xc = np.vstack([np.zeros((HALO, D), f32), x[b, :TC]])
        else:
            xc = x[b, t0 - HALO: t0 + TC]
        xTc = np.ascontiguousarray(xc.T).astype(bf)
        oh = np.zeros(NCORES, f32)
        if k > 0:
            oh[b * KCHUNKS + k - 1] = 1.0
        parts = [bias_ih_m, bias_ih_m if k > 0 else zero_halo,
                 _v2m(-np.asarray(W_gate_b, f32)), _v2m(W_decay_b),
                 _v2m(W_z_b), _v2m(conv_b), _v2m(ln_g), convw_m,
                 np.tile(oh[None, :], (128, 1))]
        if has_lb:
            parts.append(_v2m(ln_b))
        cvec_m = np.ascontiguousarray(np.concatenate(parts, axis=1)
                                      .astype(f32))
        imap = {**shared, "xT": xTc, "cvec": cvec_m}
        if ms_f8:
            imap["x8T"] = np.ascontiguousarray(xc[HALO:].T).astype(f8)
        in_maps.append(imap)

    res = run_bass_kernel_spmd(nc, in_maps, core_ids=list(range(NCORES)))

    outf = np.empty((B, T, D), f32)
    for c in range(NCORES):
        b, k = divmod(c, KCHUNKS)
        outf[b, k * TC:(k + 1) * TC, :] = res.results[c]["out"]
    return outf
